# revision 36
# baseline (speedup 1.0000x reference)
"""Trainium2 Bass kernel for a dense transformer encoder layer.

Contract: kernel(**inputs) takes FULL unsharded inputs (as produced by the
problem's setup_inputs) and returns the FULL output [B, L, D] float32.

Sharding: 8 cores, data-parallel over batch (4) x sequence-split (2).
Core c handles batch b=c//2, sequence half h=c%2 (1024 query rows), but
computes K/V over the full 2048 keys of its batch item (keys are rotated so
each core's own rows come first -> one identical SPMD program, per-core data
only). No collectives.

Host prep (layout only): fold ln1_g/ln1_b into Wqkv/bqkv, ln2_g/ln2_b into
W1/b1, fold 1/sqrt(dh) into Wq/bq, de-interleave Wqkv into [Q|K] (feature-
major outputs) and V (row-major output), cast weights to bf16.

On-chip dataflow per core (all matmuls bf16 with fp32 PSUM accumulate):
  LN1 (row-major, bn_stats) -> PE-transpose -> lnT [768, 2048] bf16
  V   = lnT.T @ Wv row-major [2048, 768(+ones col per head)]
  per head-pair j: qT/kT = Wqk.T @ lnT (feature-major)
    per head: scoresT = kT_h.T @ qT_h  (keys on partitions)
              expT = Exp(scoresT + mask_bias)   (no max-subtract; scores O(1))
              attn_outT/sumexp = [V_h|1].T @ expT  (M=65 matmul; ones col
              gives softmax denominator); scale by 1/sumexp -> aoT
  out1 = aoT.T @ Wo + bo + x (row-major, bf16)
  LN2 -> PE-transpose -> ln2T; uT = W1.T @ ln2T; z = uT.T @ W2 + b2
  out = out1 + Gelu(z)  (fp32 out)
"""

import numpy as np
import ml_dtypes

B, L, D, H, I = 4, 2048, 768, 12, 3072
DH = D // H            # 64
P = 128
LQ = L // 2            # 1024 query rows per core
NCORES = 8
EPS = 1e-5

KD = D // P            # 6   k-subtiles over D
KI = I // P            # 24  k-subtiles over I
NT = L // P            # 16  key tiles
NTQ = LQ // P          # 8   query tiles
NPAIR = H // 2         # 6   head pairs
VW = H * (DH + 1)      # 780 vaug width (64 cols + ones col per head)

_CACHE = {}


def _bf16(a):
    return np.ascontiguousarray(np.asarray(a, np.float32).astype(ml_dtypes.bfloat16))


def _f32(a):
    return np.ascontiguousarray(np.asarray(a, np.float32))


def _pm(vec, k):
    """[k*128] -> [128, k] partition-major."""
    return np.ascontiguousarray(np.asarray(vec, np.float32).reshape(k, P).T)


def _wpm(w, k):
    """[k*128, M] -> [128, k, M] partition-major lhsT/rhs layout."""
    w = np.asarray(w)
    return np.ascontiguousarray(w.reshape(k, P, w.shape[1]).transpose(1, 0, 2))


def build(use_mask=False):
    import concourse.bass as bass
    import concourse.mybir as mybir
    import concourse.tile as tile
    from concourse import bacc
    from concourse.bass import ts
    from concourse.masks import make_identity
    from contextlib import ExitStack

    f32 = mybir.dt.float32
    bf16 = mybir.dt.bfloat16
    AF = mybir.ActivationFunctionType
    OP = mybir.AluOpType

    nc = bacc.Bacc(None, target_bir_lowering=False, debug=False)

    # ---- DRAM I/O ----------------------------------------------------------
    x_d = nc.dram_tensor("xloc", [NT, P, D], f32, kind="ExternalInput")
    mb_d = nc.dram_tensor("mbias", [P, NT], f32, kind="ExternalInput")
    wqk_d = nc.dram_tensor("wqk", [P, KD, 2 * D], bf16, kind="ExternalInput")
    bqk_d = nc.dram_tensor("bqk", [P, 2 * KD], f32, kind="ExternalInput")
    wv_d = nc.dram_tensor("wv", [P, KD, D], bf16, kind="ExternalInput")
    bv_d = nc.dram_tensor("bv", [1, D], bf16, kind="ExternalInput")
    wo_d = nc.dram_tensor("wo", [P, KD, D], bf16, kind="ExternalInput")
    bo_d = nc.dram_tensor("bo", [1, D], bf16, kind="ExternalInput")
    w1_d = nc.dram_tensor("w1", [P, KD, I], bf16, kind="ExternalInput")
    b1_d = nc.dram_tensor("b1", [P, KI], f32, kind="ExternalInput")
    w2_d = nc.dram_tensor("w2", [P, KI, D], bf16, kind="ExternalInput")
    b2_d = nc.dram_tensor("b2", [1, D], bf16, kind="ExternalInput")
    out_d = nc.dram_tensor("out", [NTQ, P, D], f32, kind="ExternalOutput")

    with ExitStack() as ctx:
        tc = ctx.enter_context(tile.TileContext(nc))
        ps = ctx.enter_context(tc.tile_pool(name="ps", bufs=2, space="PSUM"))
        ps2 = ctx.enter_context(tc.tile_pool(name="ps2", bufs=3, space="PSUM"))
        const = ctx.enter_context(tc.tile_pool(name="const", bufs=1))
        wres = ctx.enter_context(tc.tile_pool(name="wres", bufs=1))
        wstr = ctx.enter_context(tc.tile_pool(name="wstr", bufs=4))
        kvp = ctx.enter_context(tc.tile_pool(name="kvp", bufs=1))
        qkt = ctx.enter_context(tc.tile_pool(name="qkt", bufs=1))
        lnu = ctx.enter_context(tc.tile_pool(name="lnu", bufs=1))
        expp = ctx.enter_context(tc.tile_pool(name="expp", bufs=3))
        xp = ctx.enter_context(tc.tile_pool(name="xp", bufs=1))
        tp = ctx.enter_context(tc.tile_pool(name="tp", bufs=2))

        nname = [0]

        def psum(cols=512, dt=f32):
            nname[0] += 1
            return ps.tile([P, cols], dt, tag="ps", name=f"ps{nname[0]}")

        def psum2():
            # two-bank psum pair [128, 2, 512] fp32
            nname[0] += 1
            return ps2.tile([P, 2, 512], f32, tag="ps2", name=f"pp{nname[0]}")

        # ---- constants -----------------------------------------------------
        ident = const.tile([P, P], bf16, tag="ident")
        make_identity(nc, ident)
        epst = const.tile([P, 1], f32, tag="eps")
        nc.vector.memset(epst, EPS)
        mbias = const.tile([P, NT], f32, tag="mb")
        nc.sync.dma_start(mbias[:], mb_d[:])
        bqk_sb = const.tile([P, 2 * KD], f32, tag="bqk")
        nc.sync.dma_start(bqk_sb[:], bqk_d[:])
        bv_sb = const.tile([P, D], bf16, tag="bv")
        nc.sync.dma_start(bv_sb[:], bv_d[:].to_broadcast((P, D)))
        bo_sb = const.tile([P, D], bf16, tag="bo")
        nc.sync.dma_start(bo_sb[:], bo_d[:].to_broadcast((P, D)))
        b1_sb = const.tile([P, KI], f32, tag="b1")
        nc.sync.dma_start(b1_sb[:], b1_d[:])
        b2_sb = const.tile([P, D], bf16, tag="b2")
        nc.sync.dma_start(b2_sb[:], b2_d[:].to_broadcast((P, D)))

        # persistent activations
        lnT = lnu.tile([P, KD, L], bf16, tag="lnu")        # [768, 2048] transposed LN1
        vaug = kvp.tile([P, NT, VW], bf16, tag="vo")       # V row-major + ones cols
        aoT = kvp.tile([P, KD, LQ], bf16, tag="aoT")       # attn out, feature-major

        def layernorm(dst_bf16, src, stats_tag):
            """dst = (src - mean)/sqrt(var+eps) over free dim 768."""
            view = src.rearrange("p (a b) -> p a b", b=256)
            stats = tp.tile([P, 3, 6], f32, tag=stats_tag + "s")
            mv = tp.tile([P, 2], f32, tag=stats_tag + "m")
            for i in range(3):
                nc.vector.bn_stats(out=stats[:, i, :], in_=view[:, i, :])
            nc.vector.bn_aggr(out=mv[:], in_=stats[:])
            # mv[:,1] = 1/sqrt(var+eps)
            nc.scalar.activation(out=mv[:, 1:2], in_=mv[:, 1:2], func=AF.Sqrt,
                                 bias=epst[:], scale=1.0)
            nc.vector.reciprocal(out=mv[:, 1:2], in_=mv[:, 1:2])
            nc.vector.tensor_scalar(out=dst_bf16, in0=src,
                                    scalar1=mv[:, 0:1], scalar2=mv[:, 1:2],
                                    op0=OP.subtract, op1=OP.mult)

        def transpose_128(dst, src_bf16):
            """dst[128,128] (sbuf bf16) = src.T via PE (XBAR DMA transpose
            measured ~3x slower here). Uses the ps2 banks, which are idle in
            the LN phases, so transposes don't contend with matmul psums."""
            nname[0] += 1
            pt = ps2.tile([P, P], bf16, tag="ps2", name=f"pt{nname[0]}")
            nc.tensor.transpose(pt[:], src_bf16, ident[:])
            nc.vector.tensor_copy(out=dst, in_=pt[:])

        # ---- Phase A: LN1 + transpose -> lnT -------------------------------
        for tpair in range(NT // 2):
            xt = xp.tile([P, 2, D], f32, tag="xl")
            if tpair == 0:
                # split the first load so LN (and the first PE transposes)
                # start ~2us earlier instead of waiting on a 1.6MB transfer
                nc.sync.dma_start(xt[:, 0, :], x_d[0])
                nc.sync.dma_start(xt[:, 1, :], x_d[1])
            else:
                nc.sync.dma_start(xt[:], x_d[2 * tpair : 2 * tpair + 2].rearrange("t p d -> p t d"))
            for s in range(2):
                t = 2 * tpair + s
                lnbf = tp.tile([P, D], bf16, tag="lnbf")
                layernorm(lnbf[:], xt[:, s, :], "ln1")
                for j in range(KD):
                    transpose_128(lnT[:, j, ts(t, P)], lnbf[:, ts(j, P)])

        # ---- Phase B0: V row-major (+ ones cols) ---------------------------
        wv_sb = wres.tile([P, KD, D], bf16, tag="wow")
        nc.sync.dma_start(wv_sb[:], wv_d[:])
        vview = vaug.rearrange("p t (h c) -> p t h c", c=DH + 1)
        nc.vector.memset(vview[:, :, :, DH : DH + 1], 1.0)
        bv3 = bv_sb.rearrange("p (h c) -> p h c", c=DH)
        for t in range(NT):
            for ncol in range(2):
                pv = psum(384)
                for k in range(KD):
                    nc.tensor.matmul(pv[:, :384], lnT[:, k, ts(t, P)],
                                     wv_sb[:, k, ts(ncol, 384)],
                                     start=(k == 0), stop=(k == KD - 1))
                dst = vview[:, t, 6 * ncol : 6 * ncol + 6, 0:DH]
                src = pv[:, :384].rearrange("p (h c) -> p h c", c=DH)
                bvb = bv3[:, 6 * ncol : 6 * ncol + 6, :]
                nc.vector.tensor_tensor(out=dst, in0=src, in1=bvb, op=OP.add)

        # ---- Phase B1+C: per head-pair QKV + attention ---------------------
        for j in range(NPAIR):
            wqkj = wstr.tile([P, KD, 2 * P], bf16, tag="wqkj")
            nc.sync.dma_start(wqkj[:, :, 0:P], wqk_d[:, :, ts(j, P)])
            nc.sync.dma_start(wqkj[:, :, P : 2 * P], wqk_d[:, :, D + j * P : D + (j + 1) * P])

            qTj = qkt.tile([P, LQ], bf16, tag="qT")
            for lch in range(2):
                pq = psum()
                for k in range(KD):
                    nc.tensor.matmul(pq[:], wqkj[:, k, 0:P], lnT[:, k, ts(lch, 512)],
                                     start=(k == 0), stop=(k == KD - 1))
                nc.vector.tensor_scalar(out=qTj[:, ts(lch, 512)], in0=pq[:],
                                        scalar1=bqk_sb[:, j : j + 1], scalar2=None,
                                        op0=OP.add)
            kTj = qkt.tile([P, L], bf16, tag="kT")
            for nch in range(4):
                pk = psum()
                for k in range(KD):
                    nc.tensor.matmul(pk[:], wqkj[:, k, P : 2 * P], lnT[:, k, ts(nch, 512)],
                                     start=(k == 0), stop=(k == KD - 1))
                nc.vector.tensor_scalar(out=kTj[:, ts(nch, 512)], in0=pk[:],
                                        scalar1=bqk_sb[:, KD + j : KD + j + 1],
                                        scalar2=None, op0=OP.add)

            for lch in range(2):
                # Heads A/B interleaved at the score-matmul level: the K=64
                # matmuls land on PE row-groups 0-63 / 64-127 and overlap.
                # ps2 bufs=3 lets PE run a step ahead of the exp drains.
                ex = [expp.tile([P, NT, 512], bf16, tag="expT",
                                name=f"ex{j}_{lch}_{hh}") for hh in range(2)]
                if use_mask:
                    for mt in range(NT):
                        for hh in range(2):
                            sc = psum()
                            nc.tensor.matmul(sc[:],
                                             kTj[hh * 64 : hh * 64 + 64, ts(mt, P)],
                                             qTj[hh * 64 : hh * 64 + 64, ts(lch, 512)],
                                             start=True, stop=True)
                            nc.scalar.activation(out=ex[hh][:, mt, :], in_=sc[:],
                                                 func=AF.Exp,
                                                 bias=mbias[:, mt : mt + 1],
                                                 scale=1.0)
                else:
                    for mtp in range(NT // 2):
                        sc2s = [psum2(), psum2()]
                        for s in range(2):
                            for hh in range(2):
                                nc.tensor.matmul(sc2s[hh][:, s, :],
                                                 kTj[hh * 64 : hh * 64 + 64,
                                                     ts(2 * mtp + s, P)],
                                                 qTj[hh * 64 : hh * 64 + 64,
                                                     ts(lch, 512)],
                                                 start=True, stop=True)
                        for hh in range(2):
                            nc.scalar.activation(
                                out=ex[hh][:, 2 * mtp : 2 * mtp + 2, :],
                                in_=sc2s[hh][:], func=AF.Exp)
                for hh in range(2):
                    h = 2 * j + hh
                    r = hh * 64
                    pvp = psum()
                    for mt in range(NT):
                        nc.tensor.matmul(pvp[0 : DH + 1, :],
                                         vaug[:, mt, h * (DH + 1) : (h + 1) * (DH + 1)],
                                         ex[hh][:, mt, :],
                                         start=(mt == 0), stop=(mt == NT - 1))
                    # 1/sumexp then replicate across the 64 head-feature
                    # partitions on GpSimd.
                    rr = tp.tile([1, 512], f32, tag="rr")
                    nc.vector.reciprocal(out=rr[:], in_=pvp[DH : DH + 1, :])
                    rrb = tp.tile([64, 512], f32, tag="rrb")
                    nc.gpsimd.partition_broadcast(rrb[:], rr[:])
                    nc.vector.tensor_tensor(out=aoT[r : r + 64, j, ts(lch, 512)],
                                            in0=pvp[0:DH, :],
                                            in1=rrb[:], op=OP.mult)

        # ---- Phase D: out-proj + residual + LN2 + transpose ----------------
        wo_sb = wres.tile([P, KD, D], bf16, tag="wow")
        nc.sync.dma_start(wo_sb[:], wo_d[:])
        out1 = kvp.tile([P, NTQ, D], bf16, tag="vo")
        ln2T = kvp.tile([P, KD, LQ], bf16, tag="ln2T")
        for t in range(NTQ):
            xr = xp.tile([P, D], f32, tag="xl")
            nc.sync.dma_start(xr[:], x_d[t].rearrange("p d -> p d"))
            for ncol in range(2):
                po = psum(384)
                for k in range(KD):
                    nc.tensor.matmul(po[:, :384], aoT[:, k, ts(t, P)],
                                     wo_sb[:, k, ts(ncol, 384)],
                                     start=(k == 0), stop=(k == KD - 1))
                tmp = tp.tile([P, 384], f32, tag="zb")
                nc.vector.tensor_tensor(out=tmp[:], in0=po[:, :384],
                                        in1=bo_sb[:, ts(ncol, 384)], op=OP.add)
                nc.vector.tensor_tensor(out=out1[:, t, ts(ncol, 384)], in0=tmp[:],
                                        in1=xr[:, ts(ncol, 384)], op=OP.add)
            lnbf = tp.tile([P, D], bf16, tag="lnbf")
            layernorm(lnbf[:], out1[:, t, :], "ln2")
            for k in range(KD):
                transpose_128(ln2T[:, k, ts(t, P)], lnbf[:, ts(k, P)])

        # ---- Phase E: FFN --------------------------------------------------
        for lch in range(2):
            uT = lnu.tile([P, KI, 512], bf16, tag="lnu")
            for mt in range(KI):
                w1t = wstr.tile([P, KD, P], bf16, tag="w1s")
                nc.sync.dma_start(w1t[:], w1_d[:, :, ts(mt, P)])
                pu = psum()
                for k in range(KD):
                    nc.tensor.matmul(pu[:], w1t[:, k, :], ln2T[:, k, ts(lch, 512)],
                                     start=(k == 0), stop=(k == KD - 1))
                nc.vector.tensor_scalar(out=uT[:, mt, :], in0=pu[:],
                                        scalar1=b1_sb[:, mt : mt + 1], scalar2=None,
                                        op0=OP.add)
            # one W2 sweep per l-chunk: 8 accumulators (4 l-tiles x 2 ncol)
            # fill all 8 PSUM banks, so W2 streams from HBM only twice total
            pza, pzb, pzc = psum2(), psum2(), psum2()
            p1 = [psum(384) for _ in range(2)]
            pz = [[p1[0][:, :384], p1[1][:, :384]],
                  [pza[:, 0, :384], pza[:, 1, :384]],
                  [pzb[:, 0, :384], pzb[:, 1, :384]],
                  [pzc[:, 0, :384], pzc[:, 1, :384]]]
            for mt in range(KI):
                w2t = wstr.tile([P, D], bf16, tag="w2s")
                nc.sync.dma_start(w2t[:], w2_d[:, mt, :])
                for tt in range(4):
                    for ncol in range(2):
                        nc.tensor.matmul(pz[tt][ncol],
                                         uT[:, mt, ts(tt, P)],
                                         w2t[:, ts(ncol, 384)],
                                         start=(mt == 0), stop=(mt == KI - 1))
            for tt in range(4):
                t = lch * 4 + tt
                osb = tp.tile([P, D], f32, tag="osb")
                for ncol in range(2):
                    zb = tp.tile([P, 384], f32, tag="zb")
                    nc.vector.tensor_tensor(out=zb[:], in0=pz[tt][ncol],
                                            in1=b2_sb[:, ts(ncol, 384)], op=OP.add)
                    gt = tp.tile([P, 384], f32, tag="gt")
                    nc.scalar.activation(out=gt[:], in_=zb[:], func=AF.Gelu)
                    nc.vector.tensor_tensor(out=osb[:, ts(ncol, 384)], in0=gt[:],
                                            in1=out1[:, t, ts(ncol, 384)], op=OP.add)
                nc.sync.dma_start(out_d[t], osb[:])

    nc.compile()
    return nc


def _prep_host(x, attention_mask, ln1_g, ln1_b, Wqkv, bqkv, Wo, bo,
               ln2_g, ln2_b, W1, b1, W2, b2):
    x = _f32(x); mask = np.asarray(attention_mask)
    ln1_g = _f32(ln1_g); ln1_b = _f32(ln1_b)
    Wqkv = _f32(Wqkv); bqkv = _f32(bqkv)
    Wo = _f32(Wo); bo = _f32(bo)
    ln2_g = _f32(ln2_g); ln2_b = _f32(ln2_b)
    W1 = _f32(W1); b1 = _f32(b1); W2 = _f32(W2); b2 = _f32(b2)

    base = np.arange(H)[:, None] * 3 * DH
    q_idx = (base + np.arange(DH)).ravel()
    k_idx = (base + DH + np.arange(DH)).ravel()
    v_idx = (base + 2 * DH + np.arange(DH)).ravel()

    scale = 1.0 / np.sqrt(DH)
    Wq = ln1_g[:, None] * Wqkv[:, q_idx] * scale
    Wk = ln1_g[:, None] * Wqkv[:, k_idx]
    Wv = ln1_g[:, None] * Wqkv[:, v_idx]
    bq = (bqkv[q_idx] + ln1_b @ Wqkv[:, q_idx]) * scale
    bk = bqkv[k_idx] + ln1_b @ Wqkv[:, k_idx]
    bv = bqkv[v_idx] + ln1_b @ Wqkv[:, v_idx]
    W1p = ln2_g[:, None] * W1
    b1p = b1 + ln2_b @ W1

    shared = {
        "wqk": _bf16(_wpm(np.concatenate([Wq, Wk], axis=1), KD)),
        "bqk": np.ascontiguousarray(
            np.concatenate([_pm(bq, KD), _pm(bk, KD)], axis=1)),
        "wv": _bf16(_wpm(Wv, KD)),
        "bv": _bf16(bv[None, :]),
        "wo": _bf16(_wpm(Wo, KD)),
        "bo": _bf16(bo[None, :]),
        "w1": _bf16(_wpm(W1p, KD)),
        "b1": _pm(b1p, KI),
        "w2": _bf16(_wpm(W2, KI)),
        "b2": _bf16(b2[None, :]),
    }

    in_maps = []
    for c in range(NCORES):
        b, half = c // 2, c % 2
        own = slice(half * LQ, (half + 1) * LQ)
        oth = slice((1 - half) * LQ, (2 - half) * LQ)
        xl = np.concatenate([x[b, own], x[b, oth]], axis=0)
        ml = np.concatenate([mask[b, own], mask[b, oth]], axis=0)
        mb = (ml.astype(np.float32) - 1.0) * 30.0
        m = dict(shared)
        m["xloc"] = np.ascontiguousarray(xl.reshape(NT, P, D))
        m["mbias"] = np.ascontiguousarray(mb.reshape(NT, P).T)
        in_maps.append(m)
    return in_maps


LAST_RESULT = None  # BassKernelResults of the most recent run (for profiling)
TRACE = False


def kernel(**inputs):
    global LAST_RESULT
    from concourse.bass_utils import run_bass_kernel_spmd

    use_mask = not bool(np.asarray(inputs["attention_mask"]).all())
    key = f"nc{int(use_mask)}"
    if key not in _CACHE:
        _CACHE[key] = build(use_mask)
    nc = _CACHE[key]

    in_maps = _prep_host(**inputs)
    res = run_bass_kernel_spmd(nc, in_maps, list(range(NCORES)), trace=TRACE)
    LAST_RESULT = res

    out = np.empty((B, L, D), np.float32)
    for c in range(NCORES):
        b, half = c // 2, c % 2
        o = res.results[c]["out"].reshape(LQ, D)
        out[b, half * LQ : (half + 1) * LQ] = o
    return out


# revision 39
# speedup vs baseline: 1.2898x; 1.2898x over previous
"""Trainium2 Bass kernel for a dense transformer encoder layer.

Contract: kernel(**inputs) takes FULL unsharded inputs (as produced by the
problem's setup_inputs) and returns the FULL output [B, L, D] float32.

Sharding: 8 cores, data-parallel over batch (4) x sequence-split (2).
Core c handles batch b=c//2, sequence half h=c%2 (1024 query rows), but
computes K/V over the full 2048 keys of its batch item (keys are rotated so
each core's own rows come first -> one identical SPMD program, per-core data
only). No collectives.

Host prep (layout only): fold ln1_g/ln1_b into Wqkv/bqkv, ln2_g/ln2_b into
W1/b1, fold 1/sqrt(dh) into Wq/bq, de-interleave Wqkv into [Q|K] (feature-
major outputs) and V (row-major output), cast weights to bf16.

On-chip dataflow per core (all matmuls bf16 with fp32 PSUM accumulate):
  LN1 (row-major, bn_stats) -> PE-transpose -> lnT [768, 2048] bf16
  V   = lnT.T @ Wv row-major [2048, 768(+ones col per head)]
  per head-pair j: qT/kT = Wqk.T @ lnT (feature-major)
    per head: scoresT = kT_h.T @ qT_h  (keys on partitions)
              expT = Exp(scoresT + mask_bias)   (no max-subtract; scores O(1))
              attn_outT/sumexp = [V_h|1].T @ expT  (M=65 matmul; ones col
              gives softmax denominator); scale by 1/sumexp -> aoT
  out1 = aoT.T @ Wo + bo + x (row-major, bf16)
  LN2 -> PE-transpose -> ln2T; uT = W1.T @ ln2T; z = uT.T @ W2 + b2
  out = out1 + Gelu(z)  (fp32 out)
"""

import numpy as np
import ml_dtypes

B, L, D, H, I = 4, 2048, 768, 12, 3072
DH = D // H            # 64
P = 128
LQ = L // 2            # 1024 query rows per core
NCORES = 8
EPS = 1e-5

KD = D // P            # 6   k-subtiles over D
KI = I // P            # 24  k-subtiles over I
NT = L // P            # 16  key tiles
NTQ = LQ // P          # 8   query tiles
NPAIR = H // 2         # 6   head pairs
VW = H * (DH + 1)      # 780 vaug width (64 cols + ones col per head)

_CACHE = {}


def _bf16(a):
    return np.ascontiguousarray(np.asarray(a, np.float32).astype(ml_dtypes.bfloat16))


def _f32(a):
    return np.ascontiguousarray(np.asarray(a, np.float32))


def _pm(vec, k):
    """[k*128] -> [128, k] partition-major."""
    return np.ascontiguousarray(np.asarray(vec, np.float32).reshape(k, P).T)


def _wpm(w, k):
    """[k*128, M] -> [128, k, M] partition-major lhsT/rhs layout."""
    w = np.asarray(w)
    return np.ascontiguousarray(w.reshape(k, P, w.shape[1]).transpose(1, 0, 2))


def build(use_mask=False):
    import concourse.bass as bass
    import concourse.mybir as mybir
    import concourse.tile as tile
    from concourse import bacc
    from concourse.bass import ts
    from concourse.masks import make_identity
    from contextlib import ExitStack

    f32 = mybir.dt.float32
    bf16 = mybir.dt.bfloat16
    AF = mybir.ActivationFunctionType
    OP = mybir.AluOpType

    nc = bacc.Bacc(None, target_bir_lowering=False, debug=False)

    # ---- DRAM I/O ----------------------------------------------------------
    x_d = nc.dram_tensor("xloc", [NT, P, D], f32, kind="ExternalInput")
    mb_d = nc.dram_tensor("mbias", [P, NT], f32, kind="ExternalInput")
    wqk_d = nc.dram_tensor("wqk", [P, KD, 2 * D], bf16, kind="ExternalInput")
    bqk_d = nc.dram_tensor("bqk", [P, 2 * KD], f32, kind="ExternalInput")
    wv_d = nc.dram_tensor("wv", [P, KD, D], bf16, kind="ExternalInput")
    bv_d = nc.dram_tensor("bv", [1, D], f32, kind="ExternalInput")
    wo_d = nc.dram_tensor("wo", [P, KD, D], bf16, kind="ExternalInput")
    bo_d = nc.dram_tensor("bo", [1, D], f32, kind="ExternalInput")
    w1_d = nc.dram_tensor("w1", [P, KD, I], bf16, kind="ExternalInput")
    b1_d = nc.dram_tensor("b1", [P, KI], f32, kind="ExternalInput")
    w2_d = nc.dram_tensor("w2", [P, KI, D], bf16, kind="ExternalInput")
    b2_d = nc.dram_tensor("b2", [1, D], f32, kind="ExternalInput")
    out_d = nc.dram_tensor("out", [NTQ, P, D], f32, kind="ExternalOutput")

    with ExitStack() as ctx:
        tc = ctx.enter_context(tile.TileContext(nc))
        ps = ctx.enter_context(tc.tile_pool(name="ps", bufs=4, space="PSUM"))
        ps2 = ctx.enter_context(tc.tile_pool(name="ps2", bufs=2, space="PSUM"))
        const = ctx.enter_context(tc.tile_pool(name="const", bufs=1))
        wres = ctx.enter_context(tc.tile_pool(name="wres", bufs=1))
        wstr = ctx.enter_context(tc.tile_pool(name="wstr", bufs=6))
        kvp = ctx.enter_context(tc.tile_pool(name="kvp", bufs=1))
        qkt = ctx.enter_context(tc.tile_pool(name="qkt", bufs=2))
        lnu = ctx.enter_context(tc.tile_pool(name="lnu", bufs=1))
        expp = ctx.enter_context(tc.tile_pool(name="expp", bufs=2))
        xp = ctx.enter_context(tc.tile_pool(name="xp", bufs=2))
        tp = ctx.enter_context(tc.tile_pool(name="tp", bufs=2))

        nname = [0]

        def psum(cols=512, dt=f32):
            nname[0] += 1
            return ps.tile([P, cols], dt, tag="ps", name=f"ps{nname[0]}")

        def psum2():
            # two-bank psum pair [128, 2, 512] fp32
            nname[0] += 1
            return ps2.tile([P, 2, 512], f32, tag="ps2", name=f"pp{nname[0]}")

        # ---- constants -----------------------------------------------------
        ident = const.tile([P, P], bf16, tag="ident")
        make_identity(nc, ident)
        epst = const.tile([P, 1], f32, tag="eps")
        nc.vector.memset(epst, EPS)
        mbias = const.tile([P, NT], f32, tag="mb")
        nc.sync.dma_start(mbias[:], mb_d[:])
        bqk_sb = const.tile([P, 2 * KD], f32, tag="bqk")
        nc.sync.dma_start(bqk_sb[:], bqk_d[:])
        bv_sb = const.tile([P, D], f32, tag="bv")
        nc.sync.dma_start(bv_sb[:], bv_d[:].to_broadcast((P, D)))
        bo_sb = const.tile([P, D], f32, tag="bo")
        nc.sync.dma_start(bo_sb[:], bo_d[:].to_broadcast((P, D)))
        b1_sb = const.tile([P, KI], f32, tag="b1")
        nc.sync.dma_start(b1_sb[:], b1_d[:])
        b2_sb = const.tile([P, D], f32, tag="b2")
        nc.sync.dma_start(b2_sb[:], b2_d[:].to_broadcast((P, D)))

        # persistent activations
        lnT = lnu.tile([P, KD, L], bf16, tag="lnu")        # [768, 2048] transposed LN1
        vaug = kvp.tile([P, NT, VW], bf16, tag="vo")       # V row-major + ones cols
        aoT = kvp.tile([P, KD, LQ], bf16, tag="aoT")       # attn out, feature-major

        def layernorm(dst_bf16, src, stats_tag):
            """dst = (src - mean)/sqrt(var+eps) over free dim 768."""
            view = src.rearrange("p (a b) -> p a b", b=256)
            stats = tp.tile([P, 3, 6], f32, tag=stats_tag + "s")
            mv = tp.tile([P, 2], f32, tag=stats_tag + "m")
            for i in range(3):
                nc.vector.bn_stats(out=stats[:, i, :], in_=view[:, i, :])
            nc.vector.bn_aggr(out=mv[:], in_=stats[:])
            # mv[:,1] = 1/sqrt(var+eps)
            nc.scalar.activation(out=mv[:, 1:2], in_=mv[:, 1:2], func=AF.Sqrt,
                                 bias=epst[:], scale=1.0)
            nc.vector.reciprocal(out=mv[:, 1:2], in_=mv[:, 1:2])
            nc.vector.tensor_scalar(out=dst_bf16, in0=src,
                                    scalar1=mv[:, 0:1], scalar2=mv[:, 1:2],
                                    op0=OP.subtract, op1=OP.mult)

        def transpose_128(dst, src_bf16):
            """dst[128,128] (sbuf bf16) = src.T via PE (XBAR DMA transpose
            measured ~3x slower here). Uses the ps2 banks, which are idle in
            the LN phases, so transposes don't contend with matmul psums."""
            nname[0] += 1
            pt = ps2.tile([P, P], bf16, tag="ps2", name=f"pt{nname[0]}")
            nc.tensor.transpose(pt[:], src_bf16, ident[:])
            nc.vector.tensor_copy(out=dst, in_=pt[:])

        # ---- Phase A: LN1 + transpose -> lnT -------------------------------
        for tpair in range(NT // 2):
            xt = xp.tile([P, 2, D], f32, tag="xl")
            if tpair == 0:
                # split the first load so LN (and the first PE transposes)
                # start ~2us earlier instead of waiting on a 1.6MB transfer
                nc.sync.dma_start(xt[:, 0, :], x_d[0])
                nc.sync.dma_start(xt[:, 1, :], x_d[1])
            else:
                nc.sync.dma_start(xt[:], x_d[2 * tpair : 2 * tpair + 2].rearrange("t p d -> p t d"))
            for s in range(2):
                t = 2 * tpair + s
                lnbf = tp.tile([P, D], bf16, tag="lnbf")
                layernorm(lnbf[:], xt[:, s, :], "ln1")
                for j in range(KD):
                    transpose_128(lnT[:, j, ts(t, P)], lnbf[:, ts(j, P)])

        # ---- Phase B0: V row-major (+ ones cols) ---------------------------
        wv_sb = wres.tile([P, KD, D], bf16, tag="wow")
        nc.sync.dma_start(wv_sb[:], wv_d[:])
        vview = vaug.rearrange("p t (h c) -> p t h c", c=DH + 1)
        nc.vector.memset(vview[:, :, :, DH : DH + 1], 1.0)
        bv3 = bv_sb.rearrange("p (h c) -> p h c", c=DH)
        for t in range(NT):
            for ncol in range(2):
                pv = psum(384)
                for k in range(KD):
                    nc.tensor.matmul(pv[:, :384], lnT[:, k, ts(t, P)],
                                     wv_sb[:, k, ts(ncol, 384)],
                                     start=(k == 0), stop=(k == KD - 1))
                dst = vview[:, t, 6 * ncol : 6 * ncol + 6, 0:DH]
                src = pv[:, :384].rearrange("p (h c) -> p h c", c=DH)
                bvb = bv3[:, 6 * ncol : 6 * ncol + 6, :]
                nc.vector.tensor_tensor(out=dst, in0=src, in1=bvb, op=OP.add)

        # ---- Phase B1+C: per head-pair QKV + attention ---------------------
        for j in range(NPAIR):
            wqkj = wstr.tile([P, KD, 2 * P], bf16, tag="wqkj")
            nc.sync.dma_start(wqkj[:, :, 0:P], wqk_d[:, :, ts(j, P)])
            nc.sync.dma_start(wqkj[:, :, P : 2 * P], wqk_d[:, :, D + j * P : D + (j + 1) * P])

            qTj = qkt.tile([P, LQ], bf16, tag="qT")
            for lch in range(2):
                pq = psum()
                for k in range(KD):
                    nc.tensor.matmul(pq[:], wqkj[:, k, 0:P], lnT[:, k, ts(lch, 512)],
                                     start=(k == 0), stop=(k == KD - 1))
                nc.vector.tensor_scalar(out=qTj[:, ts(lch, 512)], in0=pq[:],
                                        scalar1=bqk_sb[:, j : j + 1], scalar2=None,
                                        op0=OP.add)
            kTj = qkt.tile([P, L], bf16, tag="kT")
            for nch in range(4):
                pk = psum()
                for k in range(KD):
                    nc.tensor.matmul(pk[:], wqkj[:, k, P : 2 * P], lnT[:, k, ts(nch, 512)],
                                     start=(k == 0), stop=(k == KD - 1))
                nc.vector.tensor_scalar(out=kTj[:, ts(nch, 512)], in0=pk[:],
                                        scalar1=bqk_sb[:, KD + j : KD + j + 1],
                                        scalar2=None, op0=OP.add)

            for hh in range(2):
                h = 2 * j + hh
                r = hh * 64
                for lch in range(2):
                    expT = expp.tile([P, NT, 512], bf16, tag="expT",
                                     name=f"ex{j}_{hh}_{lch}")
                    if use_mask:
                        for mt in range(NT):
                            sc = psum()
                            nc.tensor.matmul(sc[:], kTj[r : r + 64, ts(mt, P)],
                                             qTj[r : r + 64, ts(lch, 512)],
                                             start=True, stop=True)
                            nc.scalar.activation(out=expT[:, mt, :], in_=sc[:],
                                                 func=AF.Exp,
                                                 bias=mbias[:, mt : mt + 1],
                                                 scale=1.0)
                    else:
                        # batch exp over 2 score tiles (halves ACT op overhead)
                        for mtp in range(NT // 2):
                            sc2 = psum2()
                            for s in range(2):
                                nc.tensor.matmul(sc2[:, s, :],
                                                 kTj[r : r + 64, ts(2 * mtp + s, P)],
                                                 qTj[r : r + 64, ts(lch, 512)],
                                                 start=True, stop=True)
                            nc.scalar.activation(
                                out=expT[:, 2 * mtp : 2 * mtp + 2, :],
                                in_=sc2[:], func=AF.Exp)
                    pvp = psum()
                    for mt in range(NT):
                        nc.tensor.matmul(pvp[0 : DH + 1, :],
                                         vaug[:, mt, h * (DH + 1) : (h + 1) * (DH + 1)],
                                         expT[:, mt, :],
                                         start=(mt == 0), stop=(mt == NT - 1))
                    # 1/sumexp then replicate across the 64 head-feature
                    # partitions on GpSimd.
                    rr = tp.tile([1, 512], f32, tag="rr")
                    nc.vector.reciprocal(out=rr[:], in_=pvp[DH : DH + 1, :])
                    rrb = tp.tile([64, 512], f32, tag="rrb")
                    nc.gpsimd.partition_broadcast(rrb[:], rr[:])
                    nc.vector.tensor_tensor(out=aoT[r : r + 64, j, ts(lch, 512)],
                                            in0=pvp[0:DH, :],
                                            in1=rrb[:], op=OP.mult)

        # ---- Phase D: out-proj + residual + LN2 + transpose ----------------
        wo_sb = wres.tile([P, KD, D], bf16, tag="wow")
        nc.sync.dma_start(wo_sb[:], wo_d[:])
        out1 = kvp.tile([P, NTQ, D], bf16, tag="vo")
        ln2T = kvp.tile([P, KD, LQ], bf16, tag="ln2T")
        for t in range(NTQ):
            xr = xp.tile([P, D], f32, tag="xl")
            nc.sync.dma_start(xr[:], x_d[t].rearrange("p d -> p d"))
            for ncol in range(2):
                po = psum(384)
                for k in range(KD):
                    nc.tensor.matmul(po[:, :384], aoT[:, k, ts(t, P)],
                                     wo_sb[:, k, ts(ncol, 384)],
                                     start=(k == 0), stop=(k == KD - 1))
                tmp = tp.tile([P, 384], f32, tag="zb")
                nc.vector.tensor_tensor(out=tmp[:], in0=po[:, :384],
                                        in1=bo_sb[:, ts(ncol, 384)], op=OP.add)
                nc.vector.tensor_tensor(out=out1[:, t, ts(ncol, 384)], in0=tmp[:],
                                        in1=xr[:, ts(ncol, 384)], op=OP.add)
            lnbf = tp.tile([P, D], bf16, tag="lnbf")
            layernorm(lnbf[:], out1[:, t, :], "ln2")
            for k in range(KD):
                transpose_128(ln2T[:, k, ts(t, P)], lnbf[:, ts(k, P)])

        # ---- Phase E: FFN --------------------------------------------------
        for lch in range(2):
            uT = lnu.tile([P, KI, 512], bf16, tag="lnu")
            for mt in range(KI):
                w1t = wstr.tile([P, KD, P], bf16, tag="w1s")
                nc.sync.dma_start(w1t[:], w1_d[:, :, ts(mt, P)])
                pu = psum()
                for k in range(KD):
                    nc.tensor.matmul(pu[:], w1t[:, k, :], ln2T[:, k, ts(lch, 512)],
                                     start=(k == 0), stop=(k == KD - 1))
                nc.vector.tensor_scalar(out=uT[:, mt, :], in0=pu[:],
                                        scalar1=b1_sb[:, mt : mt + 1], scalar2=None,
                                        op0=OP.add)
            # one W2 sweep per l-chunk: 8 accumulators (4 l-tiles x 2 ncol)
            # fill all 8 PSUM banks, so W2 streams from HBM only twice total
            pza, pzb = psum2(), psum2()
            p1 = [psum(384) for _ in range(4)]
            pz = [[p1[0][:, :384], p1[1][:, :384]],
                  [p1[2][:, :384], p1[3][:, :384]],
                  [pza[:, 0, :384], pza[:, 1, :384]],
                  [pzb[:, 0, :384], pzb[:, 1, :384]]]
            for mt in range(KI):
                w2t = wstr.tile([P, D], bf16, tag="w2s")
                nc.sync.dma_start(w2t[:], w2_d[:, mt, :])
                for tt in range(4):
                    for ncol in range(2):
                        nc.tensor.matmul(pz[tt][ncol],
                                         uT[:, mt, ts(tt, P)],
                                         w2t[:, ts(ncol, 384)],
                                         start=(mt == 0), stop=(mt == KI - 1))
            for tt in range(4):
                t = lch * 4 + tt
                osb = tp.tile([P, D], f32, tag="osb")
                for ncol in range(2):
                    zb = tp.tile([P, 384], f32, tag="zb")
                    nc.vector.tensor_tensor(out=zb[:], in0=pz[tt][ncol],
                                            in1=b2_sb[:, ts(ncol, 384)], op=OP.add)
                    gt = tp.tile([P, 384], f32, tag="gt")
                    nc.scalar.activation(out=gt[:], in_=zb[:], func=AF.Gelu)
                    nc.vector.tensor_tensor(out=osb[:, ts(ncol, 384)], in0=gt[:],
                                            in1=out1[:, t, ts(ncol, 384)], op=OP.add)
                nc.sync.dma_start(out_d[t], osb[:])

    nc.compile()
    return nc


def _prep_host(x, attention_mask, ln1_g, ln1_b, Wqkv, bqkv, Wo, bo,
               ln2_g, ln2_b, W1, b1, W2, b2):
    x = _f32(x); mask = np.asarray(attention_mask)
    ln1_g = _f32(ln1_g); ln1_b = _f32(ln1_b)
    Wqkv = _f32(Wqkv); bqkv = _f32(bqkv)
    Wo = _f32(Wo); bo = _f32(bo)
    ln2_g = _f32(ln2_g); ln2_b = _f32(ln2_b)
    W1 = _f32(W1); b1 = _f32(b1); W2 = _f32(W2); b2 = _f32(b2)

    base = np.arange(H)[:, None] * 3 * DH
    q_idx = (base + np.arange(DH)).ravel()
    k_idx = (base + DH + np.arange(DH)).ravel()
    v_idx = (base + 2 * DH + np.arange(DH)).ravel()

    scale = 1.0 / np.sqrt(DH)
    Wq = ln1_g[:, None] * Wqkv[:, q_idx] * scale
    Wk = ln1_g[:, None] * Wqkv[:, k_idx]
    Wv = ln1_g[:, None] * Wqkv[:, v_idx]
    bq = (bqkv[q_idx] + ln1_b @ Wqkv[:, q_idx]) * scale
    bk = bqkv[k_idx] + ln1_b @ Wqkv[:, k_idx]
    bv = bqkv[v_idx] + ln1_b @ Wqkv[:, v_idx]
    W1p = ln2_g[:, None] * W1
    b1p = b1 + ln2_b @ W1

    shared = {
        "wqk": _bf16(_wpm(np.concatenate([Wq, Wk], axis=1), KD)),
        "bqk": np.ascontiguousarray(
            np.concatenate([_pm(bq, KD), _pm(bk, KD)], axis=1)),
        "wv": _bf16(_wpm(Wv, KD)),
        "bv": _f32(bv[None, :]),
        "wo": _bf16(_wpm(Wo, KD)),
        "bo": _f32(bo[None, :]),
        "w1": _bf16(_wpm(W1p, KD)),
        "b1": _pm(b1p, KI),
        "w2": _bf16(_wpm(W2, KI)),
        "b2": _f32(b2[None, :]),
    }

    in_maps = []
    for c in range(NCORES):
        b, half = c // 2, c % 2
        own = slice(half * LQ, (half + 1) * LQ)
        oth = slice((1 - half) * LQ, (2 - half) * LQ)
        xl = np.concatenate([x[b, own], x[b, oth]], axis=0)
        ml = np.concatenate([mask[b, own], mask[b, oth]], axis=0)
        mb = (ml.astype(np.float32) - 1.0) * 30.0
        m = dict(shared)
        m["xloc"] = np.ascontiguousarray(xl.reshape(NT, P, D))
        m["mbias"] = np.ascontiguousarray(mb.reshape(NT, P).T)
        in_maps.append(m)
    return in_maps


LAST_RESULT = None  # BassKernelResults of the most recent run (for profiling)
TRACE = False


def kernel(**inputs):
    global LAST_RESULT
    from concourse.bass_utils import run_bass_kernel_spmd

    use_mask = not bool(np.asarray(inputs["attention_mask"]).all())
    key = f"nc{int(use_mask)}"
    if key not in _CACHE:
        _CACHE[key] = build(use_mask)
    nc = _CACHE[key]

    in_maps = _prep_host(**inputs)
    res = run_bass_kernel_spmd(nc, in_maps, list(range(NCORES)), trace=TRACE)
    LAST_RESULT = res

    out = np.empty((B, L, D), np.float32)
    for c in range(NCORES):
        b, half = c // 2, c % 2
        o = res.results[c]["out"].reshape(LQ, D)
        out[b, half * LQ : (half + 1) * LQ] = o
    return out


# revision 41
# speedup vs baseline: 1.2918x; 1.0016x over previous
"""Trainium2 Bass kernel for a dense transformer encoder layer.

Contract: kernel(**inputs) takes FULL unsharded inputs (as produced by the
problem's setup_inputs) and returns the FULL output [B, L, D] float32.

Sharding: 8 cores, data-parallel over batch (4) x sequence-split (2).
Core c handles batch b=c//2, sequence half h=c%2 (1024 query rows), but
computes K/V over the full 2048 keys of its batch item (keys are rotated so
each core's own rows come first -> one identical SPMD program, per-core data
only). No collectives.

Host prep (layout only): fold ln1_g/ln1_b into Wqkv/bqkv, ln2_g/ln2_b into
W1/b1, fold 1/sqrt(dh) into Wq/bq, de-interleave Wqkv into [Q|K] (feature-
major outputs) and V (row-major output), cast weights to bf16.

On-chip dataflow per core (all matmuls bf16 with fp32 PSUM accumulate):
  LN1 (row-major, bn_stats) -> PE-transpose -> lnT [768, 2048] bf16
  V   = lnT.T @ Wv row-major [2048, 768(+ones col per head)]
  per head-pair j: qT/kT = Wqk.T @ lnT (feature-major)
    per head: scoresT = kT_h.T @ qT_h  (keys on partitions)
              expT = Exp(scoresT + mask_bias)   (no max-subtract; scores O(1))
              attn_outT/sumexp = [V_h|1].T @ expT  (M=65 matmul; ones col
              gives softmax denominator); scale by 1/sumexp -> aoT
  out1 = aoT.T @ Wo + bo + x (row-major, bf16)
  LN2 -> PE-transpose -> ln2T; uT = W1.T @ ln2T; z = uT.T @ W2 + b2
  out = out1 + Gelu(z)  (fp32 out)
"""

import numpy as np
import ml_dtypes

B, L, D, H, I = 4, 2048, 768, 12, 3072
DH = D // H            # 64
P = 128
LQ = L // 2            # 1024 query rows per core
NCORES = 8
EPS = 1e-5

KD = D // P            # 6   k-subtiles over D
KI = I // P            # 24  k-subtiles over I
NT = L // P            # 16  key tiles
NTQ = LQ // P          # 8   query tiles
NPAIR = H // 2         # 6   head pairs
VW = H * (DH + 1)      # 780 vaug width (64 cols + ones col per head)

_CACHE = {}


def _bf16(a):
    return np.ascontiguousarray(np.asarray(a, np.float32).astype(ml_dtypes.bfloat16))


def _f32(a):
    return np.ascontiguousarray(np.asarray(a, np.float32))


def _pm(vec, k):
    """[k*128] -> [128, k] partition-major."""
    return np.ascontiguousarray(np.asarray(vec, np.float32).reshape(k, P).T)


def _wpm(w, k):
    """[k*128, M] -> [128, k, M] partition-major lhsT/rhs layout."""
    w = np.asarray(w)
    return np.ascontiguousarray(w.reshape(k, P, w.shape[1]).transpose(1, 0, 2))


def build(use_mask=False):
    import concourse.bass as bass
    import concourse.mybir as mybir
    import concourse.tile as tile
    from concourse import bacc
    from concourse.bass import ts
    from concourse.masks import make_identity
    from contextlib import ExitStack

    f32 = mybir.dt.float32
    bf16 = mybir.dt.bfloat16
    AF = mybir.ActivationFunctionType
    OP = mybir.AluOpType

    nc = bacc.Bacc(None, target_bir_lowering=False, debug=False)

    # ---- DRAM I/O ----------------------------------------------------------
    x_d = nc.dram_tensor("xloc", [NT, P, D], f32, kind="ExternalInput")
    mb_d = nc.dram_tensor("mbias", [P, NT], f32, kind="ExternalInput")
    wqk_d = nc.dram_tensor("wqk", [P, KD, 2 * D], bf16, kind="ExternalInput")
    bqk_d = nc.dram_tensor("bqk", [P, 2 * KD], f32, kind="ExternalInput")
    wv_d = nc.dram_tensor("wv", [P, KD, D], bf16, kind="ExternalInput")
    bv_d = nc.dram_tensor("bv", [1, D], f32, kind="ExternalInput")
    wo_d = nc.dram_tensor("wo", [P, KD, D], bf16, kind="ExternalInput")
    bo_d = nc.dram_tensor("bo", [1, D], f32, kind="ExternalInput")
    w1_d = nc.dram_tensor("w1", [P, KD, I], bf16, kind="ExternalInput")
    b1_d = nc.dram_tensor("b1", [P, KI], f32, kind="ExternalInput")
    w2_d = nc.dram_tensor("w2", [P, KI, D], bf16, kind="ExternalInput")
    b2_d = nc.dram_tensor("b2", [1, D], f32, kind="ExternalInput")
    out_d = nc.dram_tensor("out", [NTQ, P, D], f32, kind="ExternalOutput")
    scr_d = nc.dram_tensor("warm_scr", [P, P], f32)

    with ExitStack() as ctx:
        tc = ctx.enter_context(tile.TileContext(nc))
        ps = ctx.enter_context(tc.tile_pool(name="ps", bufs=4, space="PSUM"))
        ps2 = ctx.enter_context(tc.tile_pool(name="ps2", bufs=2, space="PSUM"))
        const = ctx.enter_context(tc.tile_pool(name="const", bufs=1))
        wres = ctx.enter_context(tc.tile_pool(name="wres", bufs=1))
        wstr = ctx.enter_context(tc.tile_pool(name="wstr", bufs=6))
        kvp = ctx.enter_context(tc.tile_pool(name="kvp", bufs=1))
        qkt = ctx.enter_context(tc.tile_pool(name="qkt", bufs=2))
        lnu = ctx.enter_context(tc.tile_pool(name="lnu", bufs=1))
        expp = ctx.enter_context(tc.tile_pool(name="expp", bufs=2))
        xp = ctx.enter_context(tc.tile_pool(name="xp", bufs=2))
        tp = ctx.enter_context(tc.tile_pool(name="tp", bufs=2))

        nname = [0]

        def psum(cols=512, dt=f32):
            nname[0] += 1
            return ps.tile([P, cols], dt, tag="ps", name=f"ps{nname[0]}")

        def psum2():
            # two-bank psum pair [128, 2, 512] fp32
            nname[0] += 1
            return ps2.tile([P, 2, 512], f32, tag="ps2", name=f"pp{nname[0]}")

        # ---- constants -----------------------------------------------------
        ident = const.tile([P, P], bf16, tag="ident")
        make_identity(nc, ident)
        epst = const.tile([P, 1], f32, tag="eps")
        nc.vector.memset(epst, EPS)
        mbias = const.tile([P, NT], f32, tag="mb")
        nc.sync.dma_start(mbias[:], mb_d[:])
        bqk_sb = const.tile([P, 2 * KD], f32, tag="bqk")
        nc.sync.dma_start(bqk_sb[:], bqk_d[:])
        bv_sb = const.tile([P, D], f32, tag="bv")
        nc.sync.dma_start(bv_sb[:], bv_d[:].to_broadcast((P, D)))
        bo_sb = const.tile([P, D], f32, tag="bo")
        nc.sync.dma_start(bo_sb[:], bo_d[:].to_broadcast((P, D)))
        b1_sb = const.tile([P, KI], f32, tag="b1")
        nc.sync.dma_start(b1_sb[:], b1_d[:])
        b2_sb = const.tile([P, D], f32, tag="b2")
        nc.sync.dma_start(b2_sb[:], b2_d[:].to_broadcast((P, D)))

        # persistent activations
        lnT = lnu.tile([P, KD, L], bf16, tag="lnu")        # [768, 2048] transposed LN1
        vaug = kvp.tile([P, NT, VW], bf16, tag="vo")       # V row-major + ones cols
        aoT = kvp.tile([P, KD, LQ], bf16, tag="aoT")       # attn out, feature-major

        def layernorm(dst_bf16, src, stats_tag):
            """dst = (src - mean)/sqrt(var+eps) over free dim 768."""
            view = src.rearrange("p (a b) -> p a b", b=256)
            stats = tp.tile([P, 3, 6], f32, tag=stats_tag + "s")
            mv = tp.tile([P, 2], f32, tag=stats_tag + "m")
            for i in range(3):
                nc.vector.bn_stats(out=stats[:, i, :], in_=view[:, i, :])
            nc.vector.bn_aggr(out=mv[:], in_=stats[:])
            # mv[:,1] = 1/sqrt(var+eps)
            nc.scalar.activation(out=mv[:, 1:2], in_=mv[:, 1:2], func=AF.Sqrt,
                                 bias=epst[:], scale=1.0)
            nc.vector.reciprocal(out=mv[:, 1:2], in_=mv[:, 1:2])
            nc.vector.tensor_scalar(out=dst_bf16, in0=src,
                                    scalar1=mv[:, 0:1], scalar2=mv[:, 1:2],
                                    op0=OP.subtract, op1=OP.mult)

        def transpose_128(dst, src_bf16):
            """dst[128,128] (sbuf bf16) = src.T via PE (XBAR DMA transpose
            measured ~3x slower here). Uses the ps2 banks, which are idle in
            the LN phases, so transposes don't contend with matmul psums."""
            nname[0] += 1
            pt = ps2.tile([P, P], bf16, tag="ps2", name=f"pt{nname[0]}")
            nc.tensor.transpose(pt[:], src_bf16, ident[:])
            nc.vector.tensor_copy(out=dst, in_=pt[:])

        # ---- Phase A: LN1 + transpose -> lnT -------------------------------
        for tpair in range(NT // 2):
            xt = xp.tile([P, 2, D], f32, tag="xl")
            if tpair == 0:
                # split the first load so LN (and the first PE transposes)
                # start ~2us earlier instead of waiting on a 1.6MB transfer
                nc.sync.dma_start(xt[:, 0, :], x_d[0])
                nc.sync.dma_start(xt[:, 1, :], x_d[1])
            else:
                nc.sync.dma_start(xt[:], x_d[2 * tpair : 2 * tpair + 2].rearrange("t p d -> p t d"))
            for s in range(2):
                t = 2 * tpair + s
                lnbf = tp.tile([P, D], bf16, tag="lnbf")
                layernorm(lnbf[:], xt[:, s, :], "ln1")
                for j in range(KD):
                    transpose_128(lnT[:, j, ts(t, P)], lnbf[:, ts(j, P)])
            if tpair == 0:
                # HAM warm-up: ~4us of real (non-transpose) matmuls early in
                # phase A flips the PE clock gate to 8/8; transposes alone
                # don't count as PE activity, so without this the whole
                # LN/V/QKV front runs at 1.2GHz (first K=8 event ~70us in).
                # DMA to scratch keeps the chain from being dead-code.
                wps = psum()
                for w in range(36):
                    nc.tensor.matmul(wps[:, 0:P], ident[:], ident[:],
                                     start=(w == 0), stop=(w == 35))
                wsb = tp.tile([P, P], f32, tag="wsb")
                nc.vector.tensor_copy(out=wsb[:], in_=wps[:, 0:P])
                nc.sync.dma_start(scr_d[:], wsb[:])

        # ---- Phase B0: V row-major (+ ones cols) ---------------------------
        wv_sb = wres.tile([P, KD, D], bf16, tag="wow")
        nc.sync.dma_start(wv_sb[:], wv_d[:])
        vview = vaug.rearrange("p t (h c) -> p t h c", c=DH + 1)
        nc.vector.memset(vview[:, :, :, DH : DH + 1], 1.0)
        bv3 = bv_sb.rearrange("p (h c) -> p h c", c=DH)
        for t in range(NT):
            for ncol in range(2):
                pv = psum(384)
                for k in range(KD):
                    nc.tensor.matmul(pv[:, :384], lnT[:, k, ts(t, P)],
                                     wv_sb[:, k, ts(ncol, 384)],
                                     start=(k == 0), stop=(k == KD - 1))
                dst = vview[:, t, 6 * ncol : 6 * ncol + 6, 0:DH]
                src = pv[:, :384].rearrange("p (h c) -> p h c", c=DH)
                bvb = bv3[:, 6 * ncol : 6 * ncol + 6, :]
                nc.vector.tensor_tensor(out=dst, in0=src, in1=bvb, op=OP.add)

        # ---- Phase B1+C: per head-pair QKV + attention ---------------------
        for j in range(NPAIR):
            wqkj = wstr.tile([P, KD, 2 * P], bf16, tag="wqkj")
            nc.sync.dma_start(wqkj[:, :, 0:P], wqk_d[:, :, ts(j, P)])
            nc.sync.dma_start(wqkj[:, :, P : 2 * P], wqk_d[:, :, D + j * P : D + (j + 1) * P])

            qTj = qkt.tile([P, LQ], bf16, tag="qT")
            for lch in range(2):
                pq = psum()
                for k in range(KD):
                    nc.tensor.matmul(pq[:], wqkj[:, k, 0:P], lnT[:, k, ts(lch, 512)],
                                     start=(k == 0), stop=(k == KD - 1))
                nc.vector.tensor_scalar(out=qTj[:, ts(lch, 512)], in0=pq[:],
                                        scalar1=bqk_sb[:, j : j + 1], scalar2=None,
                                        op0=OP.add)
            kTj = qkt.tile([P, L], bf16, tag="kT")
            for nch in range(4):
                pk = psum()
                for k in range(KD):
                    nc.tensor.matmul(pk[:], wqkj[:, k, P : 2 * P], lnT[:, k, ts(nch, 512)],
                                     start=(k == 0), stop=(k == KD - 1))
                nc.vector.tensor_scalar(out=kTj[:, ts(nch, 512)], in0=pk[:],
                                        scalar1=bqk_sb[:, KD + j : KD + j + 1],
                                        scalar2=None, op0=OP.add)

            for hh in range(2):
                h = 2 * j + hh
                r = hh * 64
                for lch in range(2):
                    expT = expp.tile([P, NT, 512], bf16, tag="expT",
                                     name=f"ex{j}_{hh}_{lch}")
                    if use_mask:
                        for mt in range(NT):
                            sc = psum()
                            nc.tensor.matmul(sc[:], kTj[r : r + 64, ts(mt, P)],
                                             qTj[r : r + 64, ts(lch, 512)],
                                             start=True, stop=True)
                            nc.scalar.activation(out=expT[:, mt, :], in_=sc[:],
                                                 func=AF.Exp,
                                                 bias=mbias[:, mt : mt + 1],
                                                 scale=1.0)
                    else:
                        # batch exp over 2 score tiles (halves ACT op overhead)
                        for mtp in range(NT // 2):
                            sc2 = psum2()
                            for s in range(2):
                                nc.tensor.matmul(sc2[:, s, :],
                                                 kTj[r : r + 64, ts(2 * mtp + s, P)],
                                                 qTj[r : r + 64, ts(lch, 512)],
                                                 start=True, stop=True)
                            nc.scalar.activation(
                                out=expT[:, 2 * mtp : 2 * mtp + 2, :],
                                in_=sc2[:], func=AF.Exp)
                    pvp = psum()
                    for mt in range(NT):
                        nc.tensor.matmul(pvp[0 : DH + 1, :],
                                         vaug[:, mt, h * (DH + 1) : (h + 1) * (DH + 1)],
                                         expT[:, mt, :],
                                         start=(mt == 0), stop=(mt == NT - 1))
                    # 1/sumexp then replicate across the 64 head-feature
                    # partitions on GpSimd.
                    rr = tp.tile([1, 512], f32, tag="rr")
                    nc.vector.reciprocal(out=rr[:], in_=pvp[DH : DH + 1, :])
                    rrb = tp.tile([64, 512], f32, tag="rrb")
                    nc.gpsimd.partition_broadcast(rrb[:], rr[:])
                    nc.vector.tensor_tensor(out=aoT[r : r + 64, j, ts(lch, 512)],
                                            in0=pvp[0:DH, :],
                                            in1=rrb[:], op=OP.mult)

        # ---- Phase D: out-proj + residual + LN2 + transpose ----------------
        wo_sb = wres.tile([P, KD, D], bf16, tag="wow")
        nc.sync.dma_start(wo_sb[:], wo_d[:])
        out1 = kvp.tile([P, NTQ, D], bf16, tag="vo")
        ln2T = kvp.tile([P, KD, LQ], bf16, tag="ln2T")
        for t in range(NTQ):
            xr = xp.tile([P, D], f32, tag="xl")
            nc.sync.dma_start(xr[:], x_d[t].rearrange("p d -> p d"))
            for ncol in range(2):
                po = psum(384)
                for k in range(KD):
                    nc.tensor.matmul(po[:, :384], aoT[:, k, ts(t, P)],
                                     wo_sb[:, k, ts(ncol, 384)],
                                     start=(k == 0), stop=(k == KD - 1))
                tmp = tp.tile([P, 384], f32, tag="zb")
                nc.vector.tensor_tensor(out=tmp[:], in0=po[:, :384],
                                        in1=bo_sb[:, ts(ncol, 384)], op=OP.add)
                nc.vector.tensor_tensor(out=out1[:, t, ts(ncol, 384)], in0=tmp[:],
                                        in1=xr[:, ts(ncol, 384)], op=OP.add)
            lnbf = tp.tile([P, D], bf16, tag="lnbf")
            layernorm(lnbf[:], out1[:, t, :], "ln2")
            for k in range(KD):
                transpose_128(ln2T[:, k, ts(t, P)], lnbf[:, ts(k, P)])

        # ---- Phase E: FFN --------------------------------------------------
        for lch in range(2):
            uT = lnu.tile([P, KI, 512], bf16, tag="lnu")
            for mt in range(KI):
                w1t = wstr.tile([P, KD, P], bf16, tag="w1s")
                nc.sync.dma_start(w1t[:], w1_d[:, :, ts(mt, P)])
                pu = psum()
                for k in range(KD):
                    nc.tensor.matmul(pu[:], w1t[:, k, :], ln2T[:, k, ts(lch, 512)],
                                     start=(k == 0), stop=(k == KD - 1))
                nc.vector.tensor_scalar(out=uT[:, mt, :], in0=pu[:],
                                        scalar1=b1_sb[:, mt : mt + 1], scalar2=None,
                                        op0=OP.add)
            # one W2 sweep per l-chunk: 8 accumulators (4 l-tiles x 2 ncol)
            # fill all 8 PSUM banks, so W2 streams from HBM only twice total
            pza, pzb = psum2(), psum2()
            p1 = [psum(384) for _ in range(4)]
            pz = [[p1[0][:, :384], p1[1][:, :384]],
                  [p1[2][:, :384], p1[3][:, :384]],
                  [pza[:, 0, :384], pza[:, 1, :384]],
                  [pzb[:, 0, :384], pzb[:, 1, :384]]]
            for mt in range(KI):
                w2t = wstr.tile([P, D], bf16, tag="w2s")
                nc.sync.dma_start(w2t[:], w2_d[:, mt, :])
                for tt in range(4):
                    for ncol in range(2):
                        nc.tensor.matmul(pz[tt][ncol],
                                         uT[:, mt, ts(tt, P)],
                                         w2t[:, ts(ncol, 384)],
                                         start=(mt == 0), stop=(mt == KI - 1))
            for tt in range(4):
                t = lch * 4 + tt
                osb = tp.tile([P, D], f32, tag="osb")
                for ncol in range(2):
                    zb = tp.tile([P, 384], f32, tag="zb")
                    nc.vector.tensor_tensor(out=zb[:], in0=pz[tt][ncol],
                                            in1=b2_sb[:, ts(ncol, 384)], op=OP.add)
                    gt = tp.tile([P, 384], f32, tag="gt")
                    nc.scalar.activation(out=gt[:], in_=zb[:], func=AF.Gelu)
                    nc.vector.tensor_tensor(out=osb[:, ts(ncol, 384)], in0=gt[:],
                                            in1=out1[:, t, ts(ncol, 384)], op=OP.add)
                nc.sync.dma_start(out_d[t], osb[:])

    nc.compile()
    return nc


def _prep_host(x, attention_mask, ln1_g, ln1_b, Wqkv, bqkv, Wo, bo,
               ln2_g, ln2_b, W1, b1, W2, b2):
    x = _f32(x); mask = np.asarray(attention_mask)
    ln1_g = _f32(ln1_g); ln1_b = _f32(ln1_b)
    Wqkv = _f32(Wqkv); bqkv = _f32(bqkv)
    Wo = _f32(Wo); bo = _f32(bo)
    ln2_g = _f32(ln2_g); ln2_b = _f32(ln2_b)
    W1 = _f32(W1); b1 = _f32(b1); W2 = _f32(W2); b2 = _f32(b2)

    base = np.arange(H)[:, None] * 3 * DH
    q_idx = (base + np.arange(DH)).ravel()
    k_idx = (base + DH + np.arange(DH)).ravel()
    v_idx = (base + 2 * DH + np.arange(DH)).ravel()

    scale = 1.0 / np.sqrt(DH)
    Wq = ln1_g[:, None] * Wqkv[:, q_idx] * scale
    Wk = ln1_g[:, None] * Wqkv[:, k_idx]
    Wv = ln1_g[:, None] * Wqkv[:, v_idx]
    bq = (bqkv[q_idx] + ln1_b @ Wqkv[:, q_idx]) * scale
    bk = bqkv[k_idx] + ln1_b @ Wqkv[:, k_idx]
    bv = bqkv[v_idx] + ln1_b @ Wqkv[:, v_idx]
    W1p = ln2_g[:, None] * W1
    b1p = b1 + ln2_b @ W1

    shared = {
        "wqk": _bf16(_wpm(np.concatenate([Wq, Wk], axis=1), KD)),
        "bqk": np.ascontiguousarray(
            np.concatenate([_pm(bq, KD), _pm(bk, KD)], axis=1)),
        "wv": _bf16(_wpm(Wv, KD)),
        "bv": _f32(bv[None, :]),
        "wo": _bf16(_wpm(Wo, KD)),
        "bo": _f32(bo[None, :]),
        "w1": _bf16(_wpm(W1p, KD)),
        "b1": _pm(b1p, KI),
        "w2": _bf16(_wpm(W2, KI)),
        "b2": _f32(b2[None, :]),
    }

    in_maps = []
    for c in range(NCORES):
        b, half = c // 2, c % 2
        own = slice(half * LQ, (half + 1) * LQ)
        oth = slice((1 - half) * LQ, (2 - half) * LQ)
        xl = np.concatenate([x[b, own], x[b, oth]], axis=0)
        ml = np.concatenate([mask[b, own], mask[b, oth]], axis=0)
        mb = (ml.astype(np.float32) - 1.0) * 30.0
        m = dict(shared)
        m["xloc"] = np.ascontiguousarray(xl.reshape(NT, P, D))
        m["mbias"] = np.ascontiguousarray(mb.reshape(NT, P).T)
        in_maps.append(m)
    return in_maps


LAST_RESULT = None  # BassKernelResults of the most recent run (for profiling)
TRACE = False


def kernel(**inputs):
    global LAST_RESULT
    from concourse.bass_utils import run_bass_kernel_spmd

    use_mask = not bool(np.asarray(inputs["attention_mask"]).all())
    key = f"nc{int(use_mask)}"
    if key not in _CACHE:
        _CACHE[key] = build(use_mask)
    nc = _CACHE[key]

    in_maps = _prep_host(**inputs)
    res = run_bass_kernel_spmd(nc, in_maps, list(range(NCORES)), trace=TRACE)
    LAST_RESULT = res

    out = np.empty((B, L, D), np.float32)
    for c in range(NCORES):
        b, half = c // 2, c % 2
        o = res.results[c]["out"].reshape(LQ, D)
        out[b, half * LQ : (half + 1) * LQ] = o
    return out
